# revision 8
# baseline (speedup 1.0000x reference)
"""KKAN Convolutional Network — Bass/Tile kernel for 8 Trainium2 NeuronCores.

Data parallel: 32 images -> 4 per core. Math reformulation:

  KANConv(3x3, 16 convs) + restore Conv2d(16->1, 3x3)
    = per-pixel features u(x) = [silu(x), B_0(x)..B_7(x)]
      -> 3x3 conv (9->16) -> 3x3 conv (16->1)

  Uniform cubic B-splines: B_j(x) = sum_k a_k relu(s-j-k)^3, s=(x+2.2)/0.4,
  a=[1,-4,6,-4,1]/6, so u is linear in 13 raw channels
  [silu(x), relu(s-m)^3] (stage 0: per-pixel 13->9 matmul, fp32 for the
  cancellation-heavy truncated powers).

  The 16-channel contraction of the restore conv folds into the first
  conv's weights: stage A' = 9 PSUM-accumulated tap-shifted bf16 matmuls
  producing 9 restore-tap partials per block; the restore conv's spatial
  shifts are applied in a final DMA rearrange + 9-way add.

Layout per core: 4 images stacked with 4 zero rows between = 1036 rows
= 14 blocks x 74 rows; each block carries 2 halo rows/cols (78 x 260).
Partition dim packs the 14 blocks (stage A': K=9*14+2pad=128, M=126).
"""
import sys
import types
import numpy as np

sys.path.insert(0, '/opt/trn_rl_repo')

GS, SO, NC16, G = 5, 3, 16, 8
HG = 2.0 / GS
T0K = -1.0 - SO * HG
A5 = np.array([1, -4, 6, -4, 1], np.float64) / 6.0
NBLK, BRI = 14, 74
RB, CB = 78, 260
NCOL = RB * CB                   # 20280
GUARD = CB + 1                   # 261
NCOLG = NCOL + 2 * GUARD         # 20802
NS = 1036
CHUNK = 1024
NCHUNK = (NCOL + CHUNK - 1) // CHUNK   # 20
CT = 512
NCT = (NCOL + CT - 1) // CT            # 40
N_CORES = 8
TAPS = [(di, dj) for di in (-1, 0, 1) for dj in (-1, 0, 1)]
CA, CBC = 7, 6                   # raw channels in MVa (silu+r0..5) / MVb (r6..11)

LAST_HW_NS = None
TRACE = False

_cached = {}


def _install_profile_hook():
    try:
        import antenv
        from trn_agent_boot.trn_boot import _ntff_profile_via_ctypes
        hooks = types.ModuleType('antenv.axon_hooks')
        h = [None]
        hooks.set_axon_ntff_profile_hook = lambda v: h.__setitem__(0, v)
        hooks.get_axon_ntff_profile_hook = lambda: h[0]
        sys.modules['antenv.axon_hooks'] = hooks
        antenv.axon_hooks = hooks
        hooks.set_axon_ntff_profile_hook(
            _ntff_profile_via_ctypes('/opt/axon/libaxon_pjrt.so'))
    except Exception:
        pass


def _zero_rows_for_block(b):
    zset = {-2, -1, NS, NS + 1}
    for i in range(3):
        zset |= set(range(260 * i + 256, 260 * i + 260))
    rows = [r for r in range(RB) if (74 * b + r - 2) in zset]
    # group into contiguous runs
    runs = []
    for r in rows:
        if runs and runs[-1][1] == r:
            runs[-1][1] = r + 1
        else:
            runs.append([r, r + 1])
    return runs


def _build():
    from concourse import bacc, tile, mybir
    f32 = mybir.dt.float32
    bf16 = mybir.dt.bfloat16
    AF = mybir.ActivationFunctionType

    nc = bacc.Bacc("TRN2", target_bir_lowering=False, debug=False)
    xha = nc.declare_dram_parameter("xha", [110, NCOL], f32, isOutput=False)
    xhb = nc.declare_dram_parameter("xhb", [84, NCOL], f32, isOutput=False)
    L0a = nc.declare_dram_parameter("L0a", [110, 126], f32, isOutput=False)
    L0b = nc.declare_dram_parameter("L0b", [84, 126], f32, isOutput=False)
    LA = nc.declare_dram_parameter("LA", [128, 9 * 128], f32, isOutput=False)
    biasa = nc.declare_dram_parameter("biasa", [84, 1], f32, isOutput=False)
    biasb = nc.declare_dram_parameter("biasb", [84, 1], f32, isOutput=False)
    rb = nc.declare_dram_parameter("rb", [128, 1], f32, isOutput=False)
    y = nc.declare_dram_parameter("y", [NBLK, BRI, 256], f32, isOutput=True)

    with tile.TileContext(nc) as tc:
        with (tc.tile_pool(name="w", bufs=1) as wp,
              tc.tile_pool(name="mv", bufs=2) as mvp,
              tc.tile_pool(name="tmp", bufs=2) as tp,
              tc.tile_pool(name="big", bufs=1) as bigp,
              tc.tile_pool(name="fin", bufs=2) as fin,
              tc.tile_pool(name="ps0", bufs=2, space="PSUM") as ps0,
              tc.tile_pool(name="psA", bufs=4, space="PSUM") as psA):

            L0at = wp.tile([110, 126], f32)
            nc.sync.dma_start(L0at[:], L0a[:])
            L0bt = wp.tile([84, 126], f32)
            nc.sync.dma_start(L0bt[:], L0b[:])
            LAf = wp.tile([128, 9 * 128], f32)
            nc.sync.dma_start(LAf[:], LA[:])
            LAt = wp.tile([128, 9 * 128], bf16)
            nc.vector.tensor_copy(LAt[:], LAf[:])
            bat = wp.tile([84, 1], f32)
            nc.sync.dma_start(bat[:], biasa[:])
            bbt = wp.tile([84, 1], f32)
            nc.sync.dma_start(bbt[:], biasb[:])
            rbt = wp.tile([128, 1], f32)
            nc.sync.dma_start(rbt[:], rb[:])

            BV = bigp.tile([128, NCOLG], bf16)
            PB = bigp.tile([126, NCOL], bf16)
            nc.vector.memset(BV[:, 0:GUARD], 0.0)
            nc.vector.memset(BV[:, GUARD + NCOL:NCOLG], 0.0)
            nc.vector.memset(BV[96:128, :], 0.0)
            zt = wp.tile([9, 4 * CB], bf16)
            nc.vector.memset(zt[:], 0.0)

            ct_done = 0
            for g in range(NCHUNK):
                c0 = g * CHUNK
                csz = min(CHUNK, NCOL - c0)
                mva = mvp.tile([110, CHUNK], f32, tag="mva")
                mvb = mvp.tile([84, CHUNK], f32, tag="mvb")
                nc.sync.dma_start(mva[:, :csz], xha[:, c0:c0 + csz])
                nc.sync.dma_start(mvb[:, :csz], xhb[:, c0:c0 + csz])
                t1a = tp.tile([84, CHUNK], f32, tag="t1a")
                t2a = tp.tile([84, CHUNK], f32, tag="t2a")
                t1b = tp.tile([84, CHUNK], f32, tag="t1b")
                t2b = tp.tile([84, CHUNK], f32, tag="t2b")
                sg = tp.tile([110, CHUNK], f32, tag="sg")
                nc.scalar.activation(t1a[:, :csz], mva[0:84, :csz], AF.Relu,
                                     bias=bat[:], scale=1.0 / HG)
                nc.scalar.activation(t2a[:, :csz], t1a[:, :csz], AF.Square)
                nc.scalar.activation(t1b[:, :csz], mvb[:, :csz], AF.Relu,
                                     bias=bbt[:], scale=1.0 / HG)
                nc.scalar.activation(t2b[:, :csz], t1b[:, :csz], AF.Square)
                nc.scalar.activation(sg[96:110, :csz], mva[96:110, :csz],
                                     AF.Sigmoid)
                nc.vector.tensor_mul(mva[0:84, :csz], t1a[:, :csz], t2a[:, :csz])
                nc.vector.tensor_mul(mvb[:, :csz], t1b[:, :csz], t2b[:, :csz])
                nc.vector.tensor_mul(mva[96:110, :csz], mva[96:110, :csz],
                                     sg[96:110, :csz])

                for i in range(0, csz, CT):
                    n = min(CT, csz - i)
                    acc0 = ps0.tile([126, CT], f32, tag="ps0")
                    nc.tensor.matmul(acc0[:, :n], L0at[:], mva[:, i:i + n],
                                     start=True, stop=False)
                    nc.tensor.matmul(acc0[:, :n], L0bt[:], mvb[:, i:i + n],
                                     start=False, stop=True)
                    nc.vector.tensor_copy(
                        BV[0:126, GUARD + c0 + i:GUARD + c0 + i + n],
                        acc0[:, :n])

                while ct_done < NCT and (
                        (ct_done + 1) * CT + GUARD <= c0 + csz
                        or g == NCHUNK - 1):
                    k0 = ct_done * CT
                    n = min(CT, NCOL - k0)
                    accA = psA.tile([126, CT], f32, tag="psA")
                    for ti, (di, dj) in enumerate(TAPS):
                        off = di * CB + dj
                        nc.tensor.matmul(
                            accA[:, :n], LAt[:, ti * 128:ti * 128 + 126],
                            BV[:, GUARD + k0 + off:GUARD + k0 + off + n],
                            start=(ti == 0), stop=(ti == 8))
                    nc.vector.tensor_copy(PB[:, k0:k0 + n], accA[:, :n])
                    ct_done += 1

            # zero the restore-conv padding halos in PB
            PBr = PB[:].rearrange("p (r c) -> p r c", r=RB, c=CB)
            nc.vector.memset(PBr[:, :, 0:2], 0.0)
            nc.vector.memset(PBr[:, :, 258:260], 0.0)
            for b in range(NBLK):
                for r0, r1 in _zero_rows_for_block(b):
                    nc.sync.dma_start(PBr[9 * b:9 * b + 9, r0:r1, :],
                                      zt[0:9, 0:(r1 - r0) * CB])

            # tap-sum: rearrange partials to row layout, 9-way add, + bias
            for b in range(NBLK):
                p2 = fin.tile([BRI, 9, 256], bf16, tag="p2")
                for ti, (di, dj) in enumerate(TAPS):
                    src = PBr[9 * b + ti:9 * b + ti + 1,
                              2 + di:2 + di + BRI, 2 + dj:2 + dj + 256]
                    nc.sync.dma_start(p2[:, ti, :], src)
                acc = fin.tile([BRI, 256], bf16, tag="acc")
                nc.vector.tensor_add(acc[:], p2[:, 0, :], p2[:, 1, :])
                for ti in range(2, 9):
                    nc.vector.tensor_add(acc[:], acc[:], p2[:, ti, :])
                ysb = fin.tile([BRI, 256], f32, tag="ysb")
                nc.scalar.activation(ysb[:], acc[:], AF.Identity,
                                     bias=rbt[0:BRI, :], scale=1.0)
                nc.sync.dma_start(y[b], ysb[:])
    nc.compile()
    return nc


def _fold_host(base_w, spline_w, spline_scaler, restore_w, restore_b):
    sw = (np.asarray(spline_w, np.float64)
          * np.asarray(spline_scaler, np.float64)[..., None])
    W1 = np.zeros((NC16, 9, 9))
    W1[:, :, 0] = base_w
    W1[:, :, 1:] = sw
    T0m = np.zeros((9, 13))
    T0m[0, 0] = 1.0
    for j in range(G):
        for k in range(5):
            T0m[1 + j, 1 + j + k] = A5[k]
    # MVa partitions: [0:84) = r_0..r_5 (6 groups of 14), [84:96) filler,
    # [96:110) = silu group.
    L0a = np.zeros((110, 126), np.float32)
    L0b = np.zeros((84, 126), np.float32)
    for b in range(NBLK):
        for c9 in range(9):
            for c in range(6):
                L0a[14 * c + b, 9 * b + c9] = T0m[c9, 1 + c]
            L0a[96 + b, 9 * b + c9] = T0m[c9, 0]
            for c in range(CBC):
                L0b[14 * c + b, 9 * b + c9] = T0m[c9, 7 + c]
    rw = np.asarray(restore_w, np.float64)[0].reshape(NC16, 9)
    comp = np.einsum('ctf,cs->tfs', W1, rw)
    LA = np.zeros((128, 9 * 128), np.float32)
    for tA in range(9):
        for b in range(NBLK):
            LA[9 * b:9 * b + 9, tA * 128 + 9 * b:tA * 128 + 9 * b + 9] = comp[tA]
    biasa = (np.float32(-T0K / HG)
             - (np.arange(84, dtype=np.float32) // 14)).reshape(84, 1)
    biasb = (np.float32(-T0K / HG) - 6
             - (np.arange(84, dtype=np.float32) // 14)).reshape(84, 1)
    rbv = np.full((128, 1), np.asarray(restore_b, np.float64)[0], np.float32)
    return L0a, L0b, LA, biasa, biasb, rbv


def _build_xh(x4):
    """x4 (4,256,256) f32 -> (xha [98,NCOL], xhb [84,NCOL])."""
    S = np.zeros((NS + 4, CB), np.float32)
    for i in range(4):
        S[2 + 260 * i:2 + 260 * i + 256, 2:258] = x4[i]
    flat = np.stack([S[74 * b:74 * b + RB] for b in range(NBLK)]
                    ).reshape(NBLK, NCOL)
    xha = np.zeros((110, NCOL), np.float32)
    xha[0:84] = np.broadcast_to(flat[None], (6, NBLK, NCOL)).reshape(84, NCOL)
    xha[96:110] = flat
    xhb = np.ascontiguousarray(
        np.broadcast_to(flat[None], (CBC, NBLK, NCOL)).reshape(84, NCOL))
    return xha, xhb


def kernel(x, base_w, spline_w, spline_scaler, restore_w, restore_b):
    global LAST_HW_NS
    from concourse.bass_utils import run_bass_kernel_spmd

    if 'nc' not in _cached:
        _install_profile_hook()
        _cached['nc'] = _build()
    nc = _cached['nc']

    x = np.asarray(x, np.float32)
    L0a, L0b, LA, biasa, biasb, rbv = _fold_host(
        base_w, spline_w, spline_scaler, restore_w, restore_b)
    in_maps = []
    for core in range(N_CORES):
        xa, xb = _build_xh(x[4 * core:4 * core + 4, 0])
        in_maps.append({"xha": xa, "xhb": xb, "L0a": L0a, "L0b": L0b,
                        "LA": LA, "biasa": biasa, "biasb": biasb, "rb": rbv})
    res = run_bass_kernel_spmd(nc, in_maps, list(range(N_CORES)), trace=TRACE)
    if res.exec_time_ns is not None:
        LAST_HW_NS = res.exec_time_ns
    out = np.empty((32, 1, 256, 256), np.float32)
    for core in range(N_CORES):
        yS = res.results[core]["y"].reshape(NS, 256)
        for i in range(4):
            out[4 * core + i, 0] = yS[260 * i:260 * i + 256]
    return out


# revision 11
# speedup vs baseline: 1.0620x; 1.0620x over previous
"""KKAN Convolutional Network — Bass/Tile kernel for 8 Trainium2 NeuronCores.

Data parallel: 32 images -> 4 per core. Math reformulation:

  KANConv(3x3, 16 convs) + restore Conv2d(16->1, 3x3)
    = per-pixel features u(x) = [silu(x), B_0(x)..B_7(x)]
      -> 3x3 conv (9->16) -> 3x3 conv (16->1)

  Uniform cubic B-splines: B_j(x) = sum_k a_k relu(s-j-k)^3, s=(x+2.2)/0.4,
  a=[1,-4,6,-4,1]/6, so u is linear in 13 raw channels
  [silu(x), relu(s-m)^3] (stage 0: per-pixel 13->9 matmul, fp32 for the
  cancellation-heavy truncated powers).

  The 16-channel contraction of the restore conv folds into the first
  conv's weights: stage A' = 9 PSUM-accumulated tap-shifted bf16 matmuls
  producing 9 restore-tap partials per block; the restore conv's spatial
  shifts are applied in a final DMA rearrange + 9-way add.

Layout per core: 4 images stacked with 4 zero rows between = 1036 rows
= 14 blocks x 74 rows; each block carries 2 halo rows/cols (78 x 260).
Partition dim packs the 14 blocks (stage A': K=9*14+2pad=128, M=126).
"""
import sys
import types
import numpy as np

sys.path.insert(0, '/opt/trn_rl_repo')

GS, SO, NC16, G = 5, 3, 16, 8
HG = 2.0 / GS
T0K = -1.0 - SO * HG
A5 = np.array([1, -4, 6, -4, 1], np.float64) / 6.0
NBLK, BRI = 14, 74
RB, CB = 78, 260
NCOL = RB * CB                   # 20280
GUARD = CB + 1                   # 261
NCOLG = NCOL + 2 * GUARD         # 20802
NS = 1036
CHUNK = 2048
NCHUNK = (NCOL + CHUNK - 1) // CHUNK   # 10
CT = 512
NCT = (NCOL + CT - 1) // CT            # 40
N_CORES = 8
TAPS = [(di, dj) for di in (-1, 0, 1) for dj in (-1, 0, 1)]
CA, CBC = 7, 6                   # raw channels in MVa (silu+r0..5) / MVb (r6..11)

LAST_HW_NS = None
TRACE = False

_cached = {}


def _install_profile_hook():
    try:
        import antenv
        from trn_agent_boot.trn_boot import _ntff_profile_via_ctypes
        hooks = types.ModuleType('antenv.axon_hooks')
        h = [None]
        hooks.set_axon_ntff_profile_hook = lambda v: h.__setitem__(0, v)
        hooks.get_axon_ntff_profile_hook = lambda: h[0]
        sys.modules['antenv.axon_hooks'] = hooks
        antenv.axon_hooks = hooks
        hooks.set_axon_ntff_profile_hook(
            _ntff_profile_via_ctypes('/opt/axon/libaxon_pjrt.so'))
    except Exception:
        pass


def _zero_rows_for_block(b):
    zset = {-2, -1, NS, NS + 1}
    for i in range(3):
        zset |= set(range(260 * i + 256, 260 * i + 260))
    rows = [r for r in range(RB) if (74 * b + r - 2) in zset]
    # group into contiguous runs
    runs = []
    for r in rows:
        if runs and runs[-1][1] == r:
            runs[-1][1] = r + 1
        else:
            runs.append([r, r + 1])
    return runs


def _build():
    from concourse import bacc, tile, mybir
    f32 = mybir.dt.float32
    bf16 = mybir.dt.bfloat16
    AF = mybir.ActivationFunctionType

    nc = bacc.Bacc("TRN2", target_bir_lowering=False, debug=False)
    xha = nc.declare_dram_parameter("xha", [110, NCOL], f32, isOutput=False)
    xhb = nc.declare_dram_parameter("xhb", [84, NCOL], f32, isOutput=False)
    L0a = nc.declare_dram_parameter("L0a", [110, 126], f32, isOutput=False)
    L0b = nc.declare_dram_parameter("L0b", [84, 126], f32, isOutput=False)
    LA = nc.declare_dram_parameter("LA", [128, 9 * 128], f32, isOutput=False)
    biasa = nc.declare_dram_parameter("biasa", [84, 1], f32, isOutput=False)
    biasb = nc.declare_dram_parameter("biasb", [84, 1], f32, isOutput=False)
    rb = nc.declare_dram_parameter("rb", [128, 1], f32, isOutput=False)
    y = nc.declare_dram_parameter("y", [NBLK, BRI, 256], f32, isOutput=True)

    with tile.TileContext(nc) as tc:
        with (tc.tile_pool(name="w", bufs=1) as wp,
              tc.tile_pool(name="mv", bufs=3) as mvp,
              tc.tile_pool(name="tmp", bufs=2) as tp,
              tc.tile_pool(name="tmp1", bufs=1) as tp1,
              tc.tile_pool(name="big", bufs=1) as bigp,
              tc.tile_pool(name="fin", bufs=2) as fin,
              tc.tile_pool(name="ps0", bufs=3, space="PSUM") as ps0,
              tc.tile_pool(name="psA", bufs=4, space="PSUM") as psA):

            L0at = wp.tile([110, 126], f32)
            nc.sync.dma_start(L0at[:], L0a[:])
            L0bt = wp.tile([84, 126], f32)
            nc.sync.dma_start(L0bt[:], L0b[:])
            LAf = wp.tile([128, 9 * 128], f32)
            nc.sync.dma_start(LAf[:], LA[:])
            LAt = wp.tile([128, 9 * 128], bf16)
            nc.vector.tensor_copy(LAt[:], LAf[:])
            bat = wp.tile([84, 1], f32)
            nc.sync.dma_start(bat[:], biasa[:])
            bbt = wp.tile([84, 1], f32)
            nc.sync.dma_start(bbt[:], biasb[:])
            rbt = wp.tile([128, 1], f32)
            nc.sync.dma_start(rbt[:], rb[:])

            BV = bigp.tile([128, NCOLG], bf16)
            PB = bigp.tile([126, NCOL], bf16)
            nc.vector.memset(BV[:, 0:GUARD], 0.0)
            nc.vector.memset(BV[:, GUARD + NCOL:NCOLG], 0.0)
            nc.gpsimd.memset(BV[96:128, :], 0.0)
            zt = wp.tile([9, 4 * CB], bf16)
            nc.vector.memset(zt[:], 0.0)

            ct_done = 0
            for g in range(NCHUNK):
                c0 = g * CHUNK
                csz = min(CHUNK, NCOL - c0)
                mva = mvp.tile([110, CHUNK], f32, tag="mva")
                mvb = mvp.tile([84, CHUNK], f32, tag="mvb")
                nc.sync.dma_start(mva[:, :csz], xha[:, c0:c0 + csz])
                nc.sync.dma_start(mvb[:, :csz], xhb[:, c0:c0 + csz])
                t1a = tp.tile([110, CHUNK], f32, tag="t1a")
                t2a = tp1.tile([84, CHUNK], f32, tag="t2a")
                t1b = tp.tile([84, CHUNK], f32, tag="t1b")
                t2b = tp1.tile([84, CHUNK], f32, tag="t2b")
                nc.scalar.activation(t1a[0:84, :csz], mva[0:84, :csz], AF.Relu,
                                     bias=bat[:], scale=1.0 / HG)
                nc.scalar.activation(t2a[:, :csz], t1a[0:84, :csz], AF.Square)
                nc.scalar.activation(t1b[:, :csz], mvb[:, :csz], AF.Relu,
                                     bias=bbt[:], scale=1.0 / HG)
                nc.scalar.activation(t2b[:, :csz], t1b[:, :csz], AF.Square)
                nc.scalar.activation(t1a[96:110, :csz], mva[96:110, :csz],
                                     AF.Sigmoid)
                nc.vector.tensor_mul(mva[0:84, :csz], t1a[0:84, :csz],
                                     t2a[:, :csz])
                nc.gpsimd.tensor_mul(mvb[:, :csz], t1b[:, :csz], t2b[:, :csz])
                nc.vector.tensor_mul(mva[96:110, :csz], mva[96:110, :csz],
                                     t1a[96:110, :csz])

                for i in range(0, csz, CT):
                    n = min(CT, csz - i)
                    acc0 = ps0.tile([126, CT], f32, tag="ps0")
                    nc.tensor.matmul(acc0[:, :n], L0at[:], mva[:, i:i + n],
                                     start=True, stop=False)
                    nc.tensor.matmul(acc0[:, :n], L0bt[:], mvb[:, i:i + n],
                                     start=False, stop=True)
                    nc.vector.tensor_copy(
                        BV[0:126, GUARD + c0 + i:GUARD + c0 + i + n],
                        acc0[:, :n])

                while ct_done < NCT and (
                        (ct_done + 1) * CT + GUARD <= c0 + csz
                        or g == NCHUNK - 1):
                    k0 = ct_done * CT
                    n = min(CT, NCOL - k0)
                    accA = psA.tile([126, CT], f32, tag="psA")
                    for ti, (di, dj) in enumerate(TAPS):
                        off = di * CB + dj
                        nc.tensor.matmul(
                            accA[:, :n], LAt[:, ti * 128:ti * 128 + 126],
                            BV[:, GUARD + k0 + off:GUARD + k0 + off + n],
                            start=(ti == 0), stop=(ti == 8))
                    nc.scalar.activation(PB[:, k0:k0 + n], accA[:, :n],
                                         AF.Copy)
                    ct_done += 1

            # zero the restore-conv padding halos in PB
            PBr = PB[:].rearrange("p (r c) -> p r c", r=RB, c=CB)
            nc.vector.memset(PBr[:, :, 0:2], 0.0)
            nc.vector.memset(PBr[:, :, 258:260], 0.0)
            for b in range(NBLK):
                for r0, r1 in _zero_rows_for_block(b):
                    nc.gpsimd.dma_start(PBr[9 * b:9 * b + 9, r0:r1, :],
                                        zt[0:9, 0:(r1 - r0) * CB])

            # tap-sum: rearrange partials to row layout, 9-way add, + bias
            dma_engines = [nc.sync, nc.gpsimd, nc.scalar]
            for b in range(NBLK):
                p2 = fin.tile([BRI, 9, 256], bf16, tag="p2")
                eng = dma_engines[b % 3]
                for ti, (di, dj) in enumerate(TAPS):
                    src = PBr[9 * b + ti:9 * b + ti + 1,
                              2 + di:2 + di + BRI, 2 + dj:2 + dj + 256]
                    eng.dma_start(p2[:, ti, :], src)
                acc = fin.tile([BRI, 256], bf16, tag="acc")
                veng = nc.vector if b % 2 == 0 else nc.gpsimd
                veng.tensor_add(acc[:], p2[:, 0, :], p2[:, 1, :])
                for ti in range(2, 9):
                    veng.tensor_add(acc[:], acc[:], p2[:, ti, :])
                ysb = fin.tile([BRI, 256], f32, tag="ysb")
                nc.scalar.activation(ysb[:], acc[:], AF.Identity,
                                     bias=rbt[0:BRI, :], scale=1.0)
                nc.sync.dma_start(y[b], ysb[:])
    nc.compile()
    return nc


def _fold_host(base_w, spline_w, spline_scaler, restore_w, restore_b):
    sw = (np.asarray(spline_w, np.float64)
          * np.asarray(spline_scaler, np.float64)[..., None])
    W1 = np.zeros((NC16, 9, 9))
    W1[:, :, 0] = base_w
    W1[:, :, 1:] = sw
    T0m = np.zeros((9, 13))
    T0m[0, 0] = 1.0
    for j in range(G):
        for k in range(5):
            T0m[1 + j, 1 + j + k] = A5[k]
    # MVa partitions: [0:84) = r_0..r_5 (6 groups of 14), [84:96) filler,
    # [96:110) = silu group.
    L0a = np.zeros((110, 126), np.float32)
    L0b = np.zeros((84, 126), np.float32)
    for b in range(NBLK):
        for c9 in range(9):
            for c in range(6):
                L0a[14 * c + b, 9 * b + c9] = T0m[c9, 1 + c]
            L0a[96 + b, 9 * b + c9] = T0m[c9, 0]
            for c in range(CBC):
                L0b[14 * c + b, 9 * b + c9] = T0m[c9, 7 + c]
    rw = np.asarray(restore_w, np.float64)[0].reshape(NC16, 9)
    comp = np.einsum('ctf,cs->tfs', W1, rw)
    LA = np.zeros((128, 9 * 128), np.float32)
    for tA in range(9):
        for b in range(NBLK):
            LA[9 * b:9 * b + 9, tA * 128 + 9 * b:tA * 128 + 9 * b + 9] = comp[tA]
    biasa = (np.float32(-T0K / HG)
             - (np.arange(84, dtype=np.float32) // 14)).reshape(84, 1)
    biasb = (np.float32(-T0K / HG) - 6
             - (np.arange(84, dtype=np.float32) // 14)).reshape(84, 1)
    rbv = np.full((128, 1), np.asarray(restore_b, np.float64)[0], np.float32)
    return L0a, L0b, LA, biasa, biasb, rbv


def _build_xh(x4):
    """x4 (4,256,256) f32 -> (xha [98,NCOL], xhb [84,NCOL])."""
    S = np.zeros((NS + 4, CB), np.float32)
    for i in range(4):
        S[2 + 260 * i:2 + 260 * i + 256, 2:258] = x4[i]
    flat = np.stack([S[74 * b:74 * b + RB] for b in range(NBLK)]
                    ).reshape(NBLK, NCOL)
    xha = np.zeros((110, NCOL), np.float32)
    xha[0:84] = np.broadcast_to(flat[None], (6, NBLK, NCOL)).reshape(84, NCOL)
    xha[96:110] = flat
    xhb = np.ascontiguousarray(
        np.broadcast_to(flat[None], (CBC, NBLK, NCOL)).reshape(84, NCOL))
    return xha, xhb


def kernel(x, base_w, spline_w, spline_scaler, restore_w, restore_b):
    global LAST_HW_NS
    from concourse.bass_utils import run_bass_kernel_spmd

    if 'nc' not in _cached:
        _install_profile_hook()
        _cached['nc'] = _build()
    nc = _cached['nc']

    x = np.asarray(x, np.float32)
    L0a, L0b, LA, biasa, biasb, rbv = _fold_host(
        base_w, spline_w, spline_scaler, restore_w, restore_b)
    in_maps = []
    for core in range(N_CORES):
        xa, xb = _build_xh(x[4 * core:4 * core + 4, 0])
        in_maps.append({"xha": xa, "xhb": xb, "L0a": L0a, "L0b": L0b,
                        "LA": LA, "biasa": biasa, "biasb": biasb, "rb": rbv})
    res = run_bass_kernel_spmd(nc, in_maps, list(range(N_CORES)), trace=TRACE)
    if res.exec_time_ns is not None:
        LAST_HW_NS = res.exec_time_ns
    out = np.empty((32, 1, 256, 256), np.float32)
    for core in range(N_CORES):
        yS = res.results[core]["y"].reshape(NS, 256)
        for i in range(4):
            out[4 * core + i, 0] = yS[260 * i:260 * i + 256]
    return out
